# revision 14
# baseline (speedup 1.0000x reference)
"""Trainium2 Bass kernel for nn_Attention_7009386627377.

Multi-head attention (16 heads, d=64) over [4, 2048, 1024] hidden states,
sharded across 8 NeuronCores as (batch b = core//2, head-group g = core%2 of
8 heads). Each core computes its disjoint [2048, 512] output slice with no
collectives; the host reassembles [4, 2048, 16, 64].

Per-core pipeline (bf16 compute, fp32 PSUM accumulation):
  hidden -> bf16 (ScalarE) -> transpose via regular matmuls -> hiddenT
  Qt/Kt = W-stationary matmuls (transposed layout), V natural (+bias, mask)
  per 4-head group step: scores as bf16 PSUM [128, 2048] (row-tiled
  concurrent pairs), ONE exp ACTIVATE N=2048 per step (scale=1/8 folded),
  ctx col-tiled pairs (V stationary), row-sums via mask-stationary 4-up
  tile-packed matmuls sharing one bank.
  Normalize after a DMA-xbar transpose using per-partition reciprocal sums.
"""
import threading

import numpy as np

B = 4
S = 2048
HID = 1024
JC = 512          # per-core qkv columns = 8 heads x 64
D = 64
N_CORES = 8

_LOCK = threading.Lock()
_CACHE = {}


def _build(s=S):
    from contextlib import ExitStack

    from concourse import bacc, mybir
    import concourse.bass as bass
    import concourse.tile as tile
    from concourse.masks import make_identity

    F32 = mybir.dt.float32
    BF16 = mybir.dt.bfloat16
    I16 = mybir.dt.int16
    EXP = mybir.ActivationFunctionType.Exp
    COPY = mybir.ActivationFunctionType.Copy
    MUL = mybir.AluOpType.mult
    ADD = mybir.AluOpType.add
    # exp2 bit-trick constants (DVE-offloaded exp):
    # i16 = round(score * 0.125*log2(e)*128 + (16256 - 0.043*128)), then
    # bitcast i16 -> bf16 gives ~exp(score/8) * (1 +/- 3% sawtooth). The
    # softmax normalization cancels the mean; residual noise ~1% on ctx.
    EXPC1 = float(0.125 * np.log2(np.e) * 128.0)
    EXPC2 = float(16256.0 - 0.043 * 128.0)

    nst = s // 128           # s-tiles
    nq = max(1, s // 512)    # 512-wide quarters of s
    qw = s // nq             # quarter width
    nkt = s // 128           # key tiles

    nc = bacc.Bacc("TRN2", target_bir_lowering=False, debug=False,
                   enable_asserts=False)

    hid = nc.dram_tensor("hidden", [s, HID], F32, kind="ExternalInput").ap()
    msk = nc.dram_tensor("mask", [s, 1], F32, kind="ExternalInput").ap()
    wq_d = nc.dram_tensor("wq", [HID, JC], F32, kind="ExternalInput").ap()
    wk_d = nc.dram_tensor("wk", [HID, JC], F32, kind="ExternalInput").ap()
    wv_d = nc.dram_tensor("wv", [HID, JC], F32, kind="ExternalInput").ap()
    bq_d = nc.dram_tensor("bq", [JC, 1], F32, kind="ExternalInput").ap()
    bk_d = nc.dram_tensor("bk", [JC, 1], F32, kind="ExternalInput").ap()
    bv_d = nc.dram_tensor("bv", [1, JC], F32, kind="ExternalInput").ap()
    out_d = nc.dram_tensor("out", [s, JC], F32, kind="ExternalOutput").ap()

    with tile.TileContext(nc) as tc, ExitStack() as ctx:
        P = ctx.enter_context
        persist = P(tc.tile_pool(name="persist", bufs=1))
        dram_pool = P(tc.tile_pool(name="dram", bufs=1, space="DRAM"))
        hstage_pool = P(tc.tile_pool(name="hstage", bufs=4))
        hbf_pool = P(tc.tile_pool(name="hbf", bufs=5))
        wstage_pool = P(tc.tile_pool(name="wstage", bufs=2))
        pt_pool = P(tc.tile_pool(name="pt", bufs=6))
        ctx_sb_pool = P(tc.tile_pool(name="ctxsb", bufs=2))
        sums_sb_pool = P(tc.tile_pool(name="sumssb", bufs=2))
        outt_pool = P(tc.tile_pool(name="outt", bufs=2))
        outf_pool = P(tc.tile_pool(name="outf", bufs=2))
        # PSUM: "big" = [128,1024] f32 scores slots (2 banks each, bufs=2),
        # "small" = [128,512] f32 slots for ctx/sums/V/transposes (4 banks).
        ps_big = P(tc.tile_pool(name="psbig", bufs=2, space="PSUM"))
        ps_small = P(tc.tile_pool(name="pssmall", bufs=4, space="PSUM"))

        ident_bf = persist.tile([128, 128], BF16, tag="ident_bf")
        make_identity(nc, ident_bf[:])
        ones_row = persist.tile([1, 128], BF16, tag="ones_row")
        nc.vector.memset(ones_row[:], 1.0)

        # hidden s-tile DMAs issue first so the PE pipeline starts early
        h_stage = []
        for t in range(nst):
            hs = hstage_pool.tile([128, HID], F32, tag="hs", name=f"hs{t}")
            nc.sync.dma_start(hs[:], hid[t * 128:(t + 1) * 128, :])
            h_stage.append(hs)

        # mask [s,1] -> [128, nst] (partition = s%128-within-tile)
        mask_sb = persist.tile([128, nst], F32, tag="mask_sb")
        for t in range(nst):
            nc.scalar.dma_start(mask_sb[:, t:t + 1],
                                msk[t * 128:(t + 1) * 128, :])
        mask_bf = persist.tile([128, nst], BF16, tag="mask_bf")
        nc.vector.tensor_copy(mask_bf[:], mask_sb[:])
        # mask column replicated 32-wide per k-tile: stationary operand of the
        # 4-up packed row-sum matmuls (M=32 per head, 4 heads per PSUM bank)
        mask_rep = persist.tile([128, nst * 32], BF16, tag="mask_rep")
        for t in range(nst):
            nc.vector.tensor_copy(mask_rep[:, t * 32:(t + 1) * 32],
                                  mask_bf[:, t:t + 1].to_broadcast([128, 32]))

        # biases: bq/bk as per-partition columns [128, 4]; bv as a row (bf16)
        bq_sb = persist.tile([128, 4], F32, tag="bq_sb")
        bk_sb = persist.tile([128, 4], F32, tag="bk_sb")
        for p in range(4):
            nc.scalar.dma_start(bq_sb[:, p:p + 1],
                                bq_d[p * 128:(p + 1) * 128, :])
            nc.scalar.dma_start(bk_sb[:, p:p + 1],
                                bk_d[p * 128:(p + 1) * 128, :])
        bv_st = wstage_pool.tile([1, JC], F32, tag="bv_st")
        nc.scalar.dma_start(bv_st[:], bv_d[:, :])
        bv_bf = persist.tile([1, JC], BF16, tag="bv_bf")
        nc.vector.tensor_copy(bv_bf[:], bv_st[:])

        # weights -> bf16 SBUF, chunked by 128 h-rows (casts on ScalarE,
        # which is otherwise idle until the first exp)
        w_sb = {}
        for wname, wd in (("wk", wk_d), ("wq", wq_d), ("wv", wv_d)):
            for hc in range(8):
                st_t = wstage_pool.tile([128, JC], F32, tag="wstage")
                nc.scalar.dma_start(st_t[:], wd[hc * 128:(hc + 1) * 128, :])
                wt = persist.tile([128, JC], BF16, tag=f"{wname}{hc}")
                nc.scalar.activation(wt[:], st_t[:], COPY)
                w_sb[(wname, hc)] = wt

        hT = [persist.tile([128, s], BF16, tag=f"hT{hc}", name=f"hT{hc}")
              for hc in range(8)]
        qT = [persist.tile([128, s], BF16, tag=f"qT{p}", name=f"qT{p}")
              for p in range(4)]
        kT = [persist.tile([128, s], BF16, tag=f"kT{p}", name=f"kT{p}")
              for p in range(4)]
        v_sb = [persist.tile([128, JC], BF16, tag=f"v{t}", name=f"v{t}")
                for t in range(nst)]
        scratch = dram_pool.tile([544, s], BF16, tag="scratch")

        zrow = persist.tile([16, 512], BF16, tag="zrow")
        nc.vector.memset(zrow[:], 0.0)
        for g in range(2):
            for zc in range(s // 512):
                nc.gpsimd.dma_start(
                    scratch[272 * g + 264:272 * g + 272,
                            zc * 512:(zc + 1) * 512], zrow[0:8, :])

        def produce_v(st):
            # V for s-tile st (+bias via K=1 matmul, mask fold on the ScalarE
            # copy); called from inside the attention stream right before ctx
            # needs it
            vp = ps_small.tile([128, JC], F32, tag="ps", name=f"vp{st}")
            for hc in range(8):
                nc.tensor.matmul(vp[:],
                                 lhsT=hT[hc][:, st * 128:(st + 1) * 128],
                                 rhs=w_sb[("wv", hc)][:],
                                 start=(hc == 0), stop=False)
            nc.tensor.matmul(vp[:], lhsT=ones_row[:], rhs=bv_bf[:],
                             start=False, stop=True)
            nc.scalar.activation(v_sb[st][:], vp[:], COPY,
                                 scale=mask_sb[:, st:st + 1])

        def produce_ht_quad(sq):
            # transposes for s-tiles 4sq..4sq+3 via REGULAR matmuls (FWL,
            # HAM-warm): per hc one [128,512] f32 psum tile holds the four
            # st transposes so the psum->hT bf16 DVE copy is one [128,512].
            hbs = []
            for j in range(4):
                hb = hbf_pool.tile([128, HID], BF16, tag="hb")
                nc.scalar.activation(hb[:], h_stage[4 * sq + j][:], COPY)
                hbs.append(hb)
            for hc in range(8):
                tp = ps_small.tile([128, 512], F32, tag="ps",
                                   name=f"tp{sq}_{hc}")
                for j in range(4):
                    nc.tensor.matmul(tp[:, j * 128:(j + 1) * 128],
                                     lhsT=hbs[j][:, hc * 128:(hc + 1) * 128],
                                     rhs=ident_bf[:], start=True, stop=True,
                                     skip_group_check=True)
                nc.vector.tensor_copy(
                    hT[hc][:, sq * 512:(sq + 1) * 512], tp[:])

        def project(dst, wname, b_sb, p, sq):
            pp = ps_small.tile([128, qw], F32, tag="ps", name=f"pp{wname}{p}_{sq}")
            for hc in range(8):
                nc.tensor.matmul(
                    pp[:], lhsT=w_sb[(wname, hc)][:, p * 128:(p + 1) * 128],
                    rhs=hT[hc][:, sq * qw:(sq + 1) * qw],
                    start=(hc == 0), stop=(hc == 7))
            nc.vector.tensor_scalar(dst[p][:, sq * qw:(sq + 1) * qw],
                                    pp[:], b_sb[:, p:p + 1], None, ADD)

        # ---- attention (4-head groups; software-pipelined kt loops) ----
        class Group:
            def __init__(g, q, r):
                g.q, g.r = q, r
                g.qs = slice(q * qw, (q + 1) * qw)
                g.pA, g.pB = 2 * r, 2 * r + 1
                g.ctxA = ps_small.tile([128, qw], F32, tag="ps",
                                       name=f"ctxA{q}_{r}")
                g.ctxB = ps_small.tile([128, qw], F32, tag="ps",
                                       name=f"ctxB{q}_{r}")
                g.sums = ps_small.tile([128, qw], F32, tag="ps",
                                       name=f"sums{q}_{r}")
                g.prev = None

            def ctx_sums(g, kt, ptA, ptB):
                # ctx heads on disjoint partition ranges (col-tiled
                # concurrent); sums 4-up packed in one bank.
                for ppp, ctx_ps, pt in ((g.pA, g.ctxA, ptA),
                                        (g.pB, g.ctxB, ptB)):
                    nc.tensor.matmul(
                        ctx_ps[0:64, :],
                        lhsT=v_sb[kt][:, ppp * 128:ppp * 128 + 64],
                        rhs=pt[:, 0:qw], start=(kt == 0),
                        stop=(kt == nkt - 1), skip_group_check=True,
                        tile_position=(0, 0))
                    nc.tensor.matmul(
                        ctx_ps[64:128, :],
                        lhsT=v_sb[kt][:, ppp * 128 + 64:ppp * 128 + 128],
                        rhs=pt[:, qw:2 * qw], start=(kt == 0),
                        stop=(kt == nkt - 1), skip_group_check=True,
                        tile_position=(0, 64))
                mrep = mask_rep[:, kt * 32:(kt + 1) * 32]
                for i, pt_half in enumerate(
                        (ptA[:, 0:qw], ptA[:, qw:2 * qw],
                         ptB[:, 0:qw], ptB[:, qw:2 * qw])):
                    nc.tensor.matmul(
                        g.sums[32 * i:32 * (i + 1), :], lhsT=mrep,
                        rhs=pt_half, start=(kt == 0),
                        stop=(kt == nkt - 1), skip_group_check=True,
                        tile_position=(0, 32 * i))

            def scores_exp(g, kt):
                # scores per head pair -> f32 [128, 1024] psum (2 banks);
                # row-tiled concurrent heads. exp on ScalarE, except every
                # 4th k-tile goes to VectorE via the exp2 bit-trick.
                ks = slice(kt * 128, (kt + 1) * 128)
                pts = []
                for ppp in (g.pA, g.pB):
                    sc = ps_big.tile([128, 2 * qw], F32, tag="big")
                    nc.tensor.matmul(sc[:, 0:qw], lhsT=kT[ppp][0:64, ks],
                                     rhs=qT[ppp][0:64, g.qs],
                                     start=True, stop=True,
                                     skip_group_check=True,
                                     tile_position=(0, 0))
                    nc.tensor.matmul(sc[:, qw:2 * qw],
                                     lhsT=kT[ppp][64:128, ks],
                                     rhs=qT[ppp][64:128, g.qs],
                                     start=True, stop=True,
                                     skip_group_check=True,
                                     tile_position=(64, 0))
                    # pair B's exp goes to VectorE (exp2 bit-trick) on odd
                    # k-tiles so ScalarE and VectorE each chew one pair
                    # concurrently; total offload fraction stays 1/4.
                    if ppp == g.pB and kt % 2 == 1:
                        pti = pt_pool.tile([128, 2 * qw], I16, tag="pti")
                        nc.vector.tensor_scalar(pti[:], sc[:], EXPC1, EXPC2,
                                                MUL, ADD)
                        pts.append(pti[:].bitcast(BF16))
                    else:
                        pt = pt_pool.tile([128, 2 * qw], BF16, tag="pt")
                        nc.scalar.activation(pt[:], sc[:], EXP, scale=0.125)
                        pts.append(pt[:])
                return pts

            def step(g, kt):
                # lag-1 software pipeline: scores/exp of kt precede
                # ctx/sums of kt-1 in the in-order engine streams
                pts = g.scores_exp(kt)
                if g.prev is not None:
                    g.ctx_sums(*g.prev)
                g.prev = (kt, pts[0], pts[1])

            def close(g):
                g.ctx_sums(*g.prev)
                q, r, qs = g.q, g.r, g.qs
                base = 272 * r
                for gi, ctx_ps in ((0, g.ctxA), (1, g.ctxB)):
                    ctx_sb = ctx_sb_pool.tile([128, qw], BF16, tag="ctxsb")
                    nc.scalar.activation(ctx_sb[:], ctx_ps[:], COPY)
                    nc.sync.dma_start(
                        scratch[base + gi * 128:base + (gi + 1) * 128, qs],
                        ctx_sb[:])
                # sums rows (partitions 0/32/64/96 = the group's 4 heads)
                # ride in scratch; the per-group xbar transposes them too
                ssb = sums_sb_pool.tile([128, qw], BF16, tag="sumssb")
                for i in range(4):
                    nc.scalar.activation(ssb[32 * i:32 * i + 1, :],
                                         g.sums[32 * i:32 * i + 1, :], COPY)
                    nc.sync.dma_start(
                        scratch[base + 256 + i:base + 257 + i, qs],
                        ssb[32 * i:32 * i + 1, :])
                for b4 in range(qw // 128):
                    sbg = q * (qw // 128) + b4
                    ot = outt_pool.tile([128, 272], BF16, tag="outt")
                    nc.sync.dma_start_transpose(
                        ot[:], scratch[base:base + 272,
                                       sbg * 128:(sbg + 1) * 128])
                    rc = persist.tile([128, 4], F32, tag=f"rc{sbg}_{r}",
                                      name=f"rc{sbg}_{r}")
                    nc.vector.reciprocal(rc[:], ot[:, 256:260])
                    of = outf_pool.tile([128, 256], F32, tag="outf")
                    for h in range(4):
                        nc.vector.tensor_scalar(
                            of[:, h * D:(h + 1) * D],
                            ot[:, h * D:(h + 1) * D],
                            rc[:, h:h + 1], None, MUL)
                    nc.sync.dma_start(
                        out_d[sbg * 128:(sbg + 1) * 128,
                              r * 256:(r + 1) * 256], of[:])

            def close_fast(g):
                # last group: transpose ctx/sums on the PE (regular matmuls
                # into a free scores slot) instead of the DMA xbar round
                # trip through DRAM — shortens the end-of-kernel tail.
                g.ctx_sums(*g.prev)
                q, r = g.q, g.r
                csA = ctx_sb_pool.tile([128, qw], BF16, tag="ctxsb")
                nc.scalar.activation(csA[:], g.ctxA[:], COPY)
                csB = ctx_sb_pool.tile([128, qw], BF16, tag="ctxsb")
                nc.vector.tensor_copy(csB[:], g.ctxB[:])
                ssb = sums_sb_pool.tile([4, qw], BF16, tag="ssb4")
                for i in range(4):
                    nc.scalar.activation(ssb[i:i + 1, :],
                                         g.sums[32 * i:32 * i + 1, :], COPY)
                for b4 in range(qw // 128):
                    sbg = q * (qw // 128) + b4
                    cs = slice(b4 * 128, (b4 + 1) * 128)
                    tpo = ps_big.tile([128, 2 * qw], F32, tag="big")
                    nc.tensor.matmul(tpo[:, 0:128], lhsT=csA[:, cs],
                                     rhs=ident_bf[:], start=True, stop=True,
                                     skip_group_check=True)
                    nc.tensor.matmul(tpo[:, 128:256], lhsT=csB[:, cs],
                                     rhs=ident_bf[:], start=True, stop=True,
                                     skip_group_check=True)
                    nc.tensor.matmul(tpo[:, 512:516], lhsT=ssb[0:4, cs],
                                     rhs=ident_bf[0:4, 0:4], start=True,
                                     stop=True, skip_group_check=True)
                    rc = persist.tile([128, 4], F32, tag=f"rcf{sbg}",
                                      name=f"rcf{sbg}")
                    nc.vector.reciprocal(rc[:], tpo[:, 512:516])
                    of = outf_pool.tile([128, 256], F32, tag="outf")
                    for h in range(4):
                        nc.vector.tensor_scalar(
                            of[:, h * D:(h + 1) * D],
                            tpo[:, (h % 2) * D + (h // 2) * 128:
                                (h % 2) * D + (h // 2) * 128 + D],
                            rc[:, h:h + 1], None, MUL)
                    nc.sync.dma_start(
                        out_d[sbg * 128:(sbg + 1) * 128,
                              r * 256:(r + 1) * 256], of[:])

        # ---- production: per quad: hiddenT then K columns (PE/DVE
        # pipelined); Q for quarter 0 last ----
        for sq in range(nq):
            produce_ht_quad(sq)
            for p in range(4):
                project(kT, "wk", bk_sb, p, sq)
        for p in range(4):
            project(qT, "wq", bq_sb, p, 0)

        g00 = Group(0, 0)
        for kt in range(nkt):
            produce_v(kt)
            g00.step(kt)
        g00.close()

        for q in range(nq):
            for r in range(2):
                if q == 0 and r == 0:
                    continue
                g = Group(q, r)
                for kt in range(nkt):
                    if r == 1 and q + 1 < nq and kt in (3, 6, 9, 12):
                        project(qT, "wq", bq_sb, kt // 3 - 1, q + 1)
                    g.step(kt)
                g.close()

    nc.compile()
    return nc


def _get_nc(s=S):
    with _LOCK:
        if s not in _CACHE:
            _CACHE[s] = _build(s)
        return _CACHE[s]


def _make_in_maps(inputs):
    hidden_states = np.asarray(inputs["hidden_states"], dtype=np.float32)
    attention_mask = np.asarray(inputs["attention_mask"], dtype=np.float32)
    Wq = np.asarray(inputs["Wq"], dtype=np.float32)
    Wk = np.asarray(inputs["Wk"], dtype=np.float32)
    Wv = np.asarray(inputs["Wv"], dtype=np.float32)
    bq = np.asarray(inputs["bq"], dtype=np.float32)
    bk = np.asarray(inputs["bk"], dtype=np.float32)
    bv = np.asarray(inputs["bv"], dtype=np.float32)

    in_maps = []
    for core in range(N_CORES):
        b, g = core // 2, core % 2
        js = slice(g * JC, (g + 1) * JC)
        in_maps.append({
            "hidden": np.ascontiguousarray(hidden_states[b]),
            "mask": np.ascontiguousarray(attention_mask[b].reshape(S, 1)),
            "wq": np.ascontiguousarray(Wq[:, js]),
            "wk": np.ascontiguousarray(Wk[:, js]),
            "wv": np.ascontiguousarray(Wv[:, js]),
            "bq": np.ascontiguousarray(bq[js].reshape(JC, 1)),
            "bk": np.ascontiguousarray(bk[js].reshape(JC, 1)),
            "bv": np.ascontiguousarray(bv[js].reshape(1, JC)),
        })
    return in_maps


def kernel(hidden_states, attention_mask, Wq, bq, Wk, bk, Wv, bv):
    from concourse.bass_utils import run_bass_kernel_spmd

    nc = _get_nc()
    in_maps = _make_in_maps(dict(
        hidden_states=hidden_states, attention_mask=attention_mask,
        Wq=Wq, bq=bq, Wk=Wk, bk=bk, Wv=Wv, bv=bv))

    res = run_bass_kernel_spmd(nc, in_maps, core_ids=list(range(N_CORES)))
    out = np.empty((B, S, 16, D), dtype=np.float32)
    for core in range(N_CORES):
        b, g = core // 2, core % 2
        out[b, :, g * 8:(g + 1) * 8, :] = \
            res.results[core]["out"].reshape(S, 8, D)
    return out
